# revision 11
# baseline (speedup 1.0000x reference)
"""Expert-parallel MoE SwiGLU kernel for Trainium2 (8 NeuronCores).

Strategy: each of the 8 cores owns one expert's weights (w1/w3/w2).  Token
routing (the "all-to-all dispatch") is done host-side: tokens are gathered
per expert, padded to a common capacity T, and each core computes

    y_e = (silu(x_e @ w1_e) * (x_e @ w3_e)) @ w2_e          # [T, H]

for its expert's token set.  The host then scatter-adds the weighted
per-expert outputs back into the [B, H] result.  Matmuls run in float32r
(full-rate fp32 mode on the PE array); all data stays fp32 end to end.
"""

import numpy as np

_P = 128
_E = 8  # experts == cores

# (H, I, T) -> compiled Bass program
_PROG_CACHE = {}
# test hooks: set TRACE=True before calling kernel() to capture an NTFF
# profile; the BassKernelResults of the last run lands in LAST_RUN.
TRACE = False
LAST_RUN = None


def _build_program(H, I, T):
    import concourse.bass as bass
    import concourse.tile as tile
    from concourse import bacc, mybir

    f32 = mybir.dt.float32
    f32r = mybir.dt.float32r
    Sigmoid = mybir.ActivationFunctionType.Sigmoid
    ts = bass.ts

    HC = H // _P
    IC = I // _P
    TC = T // _P
    HTILE = 512
    HT = H // HTILE
    assert H % HTILE == 0 and I % _P == 0 and T % _P == 0

    # phase-1 free-dim tiling: chunks of <=512, as equal as possible (so the
    # tail chunk stays >=256 and f32r matmuls keep their 1 cycle/row rate)
    nt = -(-T // 512)
    q, r = divmod(T, nt)
    fsz = [q + (1 if i < r else 0) for i in range(nt)]
    # phase-2 t-chunk grouping: at most 6 PSUM banks of accumulators at once
    ng = -(-TC // 6)
    gq, gr = divmod(TC, ng)
    gsz = [gq + (1 if i < gr else 0) for i in range(ng)]

    nc = bacc.Bacc(
        "TRN2",
        target_bir_lowering=False,
        debug=False,
        enable_asserts=False,
        num_devices=_E,
    )
    # inputs are declared float32r (same bits as fp32 on the numpy side) so
    # the BIR verifier sees a consistent f32r producer chain into the
    # full-rate f32r matmuls
    xT = nc.dram_tensor("xT", [H, T], f32r, kind="ExternalInput").ap()
    w1 = nc.dram_tensor("w1", [H, I], f32r, kind="ExternalInput").ap()
    w3 = nc.dram_tensor("w3", [H, I], f32r, kind="ExternalInput").ap()
    w2 = nc.dram_tensor("w2", [I, H], f32r, kind="ExternalInput").ap()
    y = nc.dram_tensor("y", [T, H], f32, kind="ExternalOutput").ap()

    w_bufs = 2
    w2_bufs = 12 if T <= 640 else 8

    with tile.TileContext(nc) as tc:
        with (
            tc.tile_pool(name="xp", bufs=1) as xp,
            tc.tile_pool(name="cp", bufs=1) as cp,
            tc.tile_pool(name="wp", bufs=w_bufs) as wp,
            tc.tile_pool(name="w2p", bufs=w2_bufs) as w2p,
            tc.tile_pool(name="hp", bufs=1) as hp,
            tc.tile_pool(name="sp", bufs=3) as sp,
            tc.tile_pool(name="op", bufs=4) as op,
            tc.tile_pool(name="pp", bufs=8, space="PSUM") as pp,
        ):
            zbias = cp.tile([_P, 1], f32)
            nc.any.memset(zbias[:], 0.0)

            # resident activations: x^T as [p, hc, t], h^T as [p, ic, t].
            # x loads as 4 chunks spread over different engine queues so the
            # first chunks land fast and the first accumulation group can
            # start without waiting for the whole 4MB.
            xTr = xT.rearrange("(hc p) t -> p hc t", p=_P)
            n_xc = 4 if HC % 4 == 0 else (2 if HC % 2 == 0 else 1)
            xcs = HC // n_xc
            x_engs = [nc.sync, nc.scalar, nc.gpsimd, nc.gpsimd]
            xs_chunks = []
            for c in range(n_xc):
                xc = xp.tile([_P, xcs, T], f32r, tag=f"xs{c}", name=f"xs_{c}")
                x_engs[c % 4].dma_start(xc[:], xTr[:, c * xcs : (c + 1) * xcs, :])
                xs_chunks.append(xc)

            def xs_slice(hc, lo, hi):
                return xs_chunks[hc // xcs][:, hc % xcs, lo:hi]

            hs = hp.tile([_P, IC, T], f32r)

            w1r = w1.rearrange("(hc p) i -> p hc i", p=_P)
            w3r = w3.rearrange("(hc p) i -> p hc i", p=_P)

            # ---- phase 1: h^T[i, t] = silu(w1^T x)[i, t] * (w3^T x)[i, t]
            for ic in range(IC):
                w1s = wp.tile([_P, HC, _P], f32r, tag="w1", name=f"w1s_{ic}")
                nc.sync.dma_start(w1s[:], w1r[:, :, ts(ic, _P)])
                w3s = wp.tile([_P, HC, _P], f32r, tag="w3", name=f"w3s_{ic}")
                nc.scalar.dma_start(w3s[:], w3r[:, :, ts(ic, _P)])
                off = 0
                for ti, ft in enumerate(fsz):
                    pg = pp.tile([_P, 512], f32, tag="ps", name=f"pg_{ic}_{ti}")
                    pu = pp.tile([_P, 512], f32, tag="ps", name=f"pu_{ic}_{ti}")
                    for hc in range(HC):
                        nc.tensor.matmul(
                            pg[:, :ft],
                            lhsT=w1s[:, hc, :],
                            rhs=xs_slice(hc, off, off + ft),
                            start=(hc == 0),
                            stop=(hc == HC - 1),
                        )
                    for hc in range(HC):
                        nc.tensor.matmul(
                            pu[:, :ft],
                            lhsT=w3s[:, hc, :],
                            rhs=xs_slice(hc, off, off + ft),
                            start=(hc == 0),
                            stop=(hc == HC - 1),
                        )
                    # silu(g) * u  ==  sigmoid(g) * g * u
                    sig = sp.tile([_P, 512], f32, tag="sig", name=f"sig_{ic}_{ti}")
                    nc.scalar.activation(sig[:, :ft], pg[:, :ft], Sigmoid, bias=zbias[:])
                    gs = sp.tile([_P, 512], f32, tag="gs", name=f"gs_{ic}_{ti}")
                    nc.vector.tensor_mul(gs[:, :ft], sig[:, :ft], pg[:, :ft])
                    nc.vector.tensor_mul(
                        hs[:, ic, off : off + ft], gs[:, :ft], pu[:, :ft]
                    )
                    off += ft

            # ---- phase 2: y[t, h] = sum_i h^T[i, t] * w2[i, h]
            for ht in range(HT):
                t0 = 0
                for g, gn in enumerate(gsz):
                    ys = [
                        pp.tile([_P, HTILE], f32, tag="ps", name=f"y_{ht}_{g}_{j}")
                        for j in range(gn)
                    ]
                    for ic in range(IC):
                        w2s = w2p.tile(
                            [_P, HTILE], f32r, tag="w2", name=f"w2s_{ht}_{g}_{ic}"
                        )
                        # alternate between the two HWDGE rings
                        dma_eng = nc.sync if ic % 2 == 0 else nc.scalar
                        dma_eng.dma_start(w2s[:], w2[ts(ic, _P), ts(ht, HTILE)])
                        for j in range(gn):
                            nc.tensor.matmul(
                                ys[j][:],
                                lhsT=hs[:, ic, ts(t0 + j, _P)],
                                rhs=w2s[:],
                                start=(ic == 0),
                                stop=(ic == IC - 1),
                            )
                    for j in range(gn):
                        ot = op.tile([_P, HTILE], f32, tag="ot", name=f"ot_{ht}_{g}_{j}")
                        nc.vector.tensor_copy(ot[:], ys[j][:])
                        nc.scalar.dma_start(y[ts(t0 + j, _P), ts(ht, HTILE)], ot[:])
                    t0 += gn

    nc.compile()
    return nc


def _get_program(H, I, T):
    key = (H, I, T)
    if key not in _PROG_CACHE:
        _PROG_CACHE[key] = _build_program(H, I, T)
    return _PROG_CACHE[key]


def kernel(x, expert_indices, expert_weights, w1, w2, w3):
    global LAST_RUN
    from concourse.bass_utils import run_bass_kernel_spmd

    x = np.ascontiguousarray(np.asarray(x, dtype=np.float32))
    idx = np.asarray(expert_indices)
    idx_dtype = idx.dtype
    idx = idx.astype(np.int64)
    wts = np.asarray(expert_weights, dtype=np.float32)
    w1 = np.asarray(w1, dtype=np.float32)
    w2 = np.asarray(w2, dtype=np.float32)
    w3 = np.asarray(w3, dtype=np.float32)

    B, H = x.shape
    E, _, I = w1.shape
    assert E == _E, f"expected {_E} experts, got {E}"
    K = idx.shape[1]

    # host-side dispatch: per-token expert weight matrix (merges duplicate
    # top-k hits of the same expert), then token lists per expert
    wmat = np.zeros((B, E), np.float32)
    np.add.at(wmat, (np.arange(B)[:, None], idx), wts)
    sel = np.zeros((B, E), bool)
    sel[np.arange(B)[:, None], idx] = True

    toks = [np.nonzero(sel[:, e])[0] for e in range(E)]
    max_count = max(len(t) for t in toks)

    # capacity per round: SBUF residency (x^T and h^T tiles) caps T at 768
    cap_limit = 768
    rounds = max(1, -(-max_count // cap_limit))
    per_round = -(-max_count // rounds)
    T = max(_P, -(-per_round // _P) * _P)

    nc = _get_program(H, I, T)
    xTfull = np.ascontiguousarray(x.T)  # [H, B]

    out = np.zeros((B, H), np.float32)
    for rd in range(rounds):
        in_maps = []
        rtoks = []
        for e in range(E):
            te = toks[e][rd * per_round : (rd + 1) * per_round]
            rtoks.append(te)
            xTe = np.zeros((H, T), np.float32)
            if len(te):
                xTe[:, : len(te)] = xTfull[:, te]
            in_maps.append(
                {
                    "xT": xTe,
                    "w1": np.ascontiguousarray(w1[e]),
                    "w3": np.ascontiguousarray(w3[e]),
                    "w2": np.ascontiguousarray(w2[e]),
                }
            )
        res = run_bass_kernel_spmd(nc, in_maps, list(range(_E)), trace=TRACE)
        LAST_RUN = res
        for e in range(E):
            te = rtoks[e]
            if len(te):
                ye = res.results[e]["y"][: len(te)]
                out[te] += wmat[te, e][:, None] * ye

    return out


# revision 17
# speedup vs baseline: 1.1738x; 1.1738x over previous
"""Expert-parallel MoE SwiGLU kernel for Trainium2 (8 NeuronCores).

Strategy: each of the 8 cores owns one expert's weights (w1/w3/w2).  Token
routing (the "all-to-all dispatch") is done host-side: tokens are gathered
per expert, padded to a common capacity T, and each core computes

    y_e = (silu(x_e @ w1_e) * (x_e @ w3_e)) @ w2_e          # [T, H]

for its expert's token set.  The host then scatter-adds the weighted
per-expert outputs back into the [B, H] result.  Matmuls run in float32r
(full-rate fp32 mode on the PE array); all data stays fp32 end to end.
"""

import numpy as np

_P = 128
_E = 8  # experts == cores

# (H, I, T) -> compiled Bass program
_PROG_CACHE = {}
# test hooks: set TRACE=True before calling kernel() to capture an NTFF
# profile; the BassKernelResults of the last run lands in LAST_RUN.
TRACE = False
LAST_RUN = None


def _build_program(H, I, T):
    import concourse.bass as bass
    import concourse.tile as tile
    from concourse import bacc, mybir

    f32 = mybir.dt.float32
    f32r = mybir.dt.float32r
    Sigmoid = mybir.ActivationFunctionType.Sigmoid
    ts = bass.ts

    HC = H // _P
    IC = I // _P
    assert H % _P == 0 and I % _P == 0 and T % 16 == 0

    # token free-dim tiling (both phases): chunks of <=512, as equal as
    # possible (so chunks stay >=256 and f32r matmuls keep 1 cycle/row)
    nt = -(-T // 512)
    q, r = divmod(T, nt)
    fsz = [q + (1 if i < r else 0) for i in range(nt)]
    foff = [sum(fsz[:i]) for i in range(nt)]

    nc = bacc.Bacc(
        "TRN2",
        target_bir_lowering=False,
        debug=False,
        enable_asserts=False,
        num_devices=_E,
    )
    # inputs are declared float32r (same bits as fp32 on the numpy side) so
    # the BIR verifier sees a consistent f32r producer chain into the
    # full-rate f32r matmuls
    xT = nc.dram_tensor("xT", [H, T], f32r, kind="ExternalInput").ap()
    w1 = nc.dram_tensor("w1", [H, I], f32r, kind="ExternalInput").ap()
    w3 = nc.dram_tensor("w3", [H, I], f32r, kind="ExternalInput").ap()
    w2 = nc.dram_tensor("w2", [I, H], f32r, kind="ExternalInput").ap()
    # output is y^T [H, T]: phase 2 accumulates with H on partitions so the
    # token dim needs no 128-granularity (capacity T can hug max_count)
    y = nc.dram_tensor("y", [H, T], f32, kind="ExternalOutput").ap()

    w_bufs = 2
    w2_bufs = 3

    with tile.TileContext(nc) as tc:
        with (
            tc.tile_pool(name="xp", bufs=1) as xp,
            tc.tile_pool(name="cp", bufs=1) as cp,
            tc.tile_pool(name="wp", bufs=w_bufs) as wp,
            tc.tile_pool(name="w2p", bufs=w2_bufs) as w2p,
            tc.tile_pool(name="hp", bufs=1) as hp,
            tc.tile_pool(name="sp", bufs=3) as sp,
            tc.tile_pool(name="op", bufs=4) as op,
            tc.tile_pool(name="pp", bufs=8, space="PSUM") as pp,
        ):
            zbias = cp.tile([_P, 1], f32)
            nc.any.memset(zbias[:], 0.0)

            # resident activations: x^T as [p, hc, t], h^T as [p, ic, t].
            # x loads as 4 chunks spread over different engine queues so the
            # first chunks land fast and the first accumulation group can
            # start without waiting for the whole 4MB.
            xTr = xT.rearrange("(hc p) t -> p hc t", p=_P)
            n_xc = 4 if HC % 4 == 0 else (2 if HC % 2 == 0 else 1)
            xcs = HC // n_xc
            x_engs = [nc.sync, nc.scalar, nc.gpsimd, nc.gpsimd]
            xs_chunks = []
            for c in range(n_xc):
                xc = xp.tile([_P, xcs, T], f32r, tag=f"xs{c}", name=f"xs_{c}")
                x_engs[c % 4].dma_start(xc[:], xTr[:, c * xcs : (c + 1) * xcs, :])
                xs_chunks.append(xc)

            def xs_slice(hc, lo, hi):
                return xs_chunks[hc // xcs][:, hc % xcs, lo:hi]

            hs = hp.tile([_P, IC, T], f32r)

            w1r = w1.rearrange("(hc p) i -> p hc i", p=_P)
            w3r = w3.rearrange("(hc p) i -> p hc i", p=_P)

            # ---- phase 1: h^T[i, t] = silu(w1^T x)[i, t] * (w3^T x)[i, t]
            for ic in range(IC):
                w1s = wp.tile([_P, HC, _P], f32r, tag="w1", name=f"w1s_{ic}")
                nc.sync.dma_start(w1s[:], w1r[:, :, ts(ic, _P)])
                w3s = wp.tile([_P, HC, _P], f32r, tag="w3", name=f"w3s_{ic}")
                nc.scalar.dma_start(w3s[:], w3r[:, :, ts(ic, _P)])
                for ti, (off, ft) in enumerate(zip(foff, fsz)):
                    pg = pp.tile([_P, 512], f32, tag="ps", name=f"pg_{ic}_{ti}")
                    pu = pp.tile([_P, 512], f32, tag="ps", name=f"pu_{ic}_{ti}")
                    for hc in range(HC):
                        nc.tensor.matmul(
                            pg[:, :ft],
                            lhsT=w1s[:, hc, :],
                            rhs=xs_slice(hc, off, off + ft),
                            start=(hc == 0),
                            stop=(hc == HC - 1),
                        )
                    for hc in range(HC):
                        nc.tensor.matmul(
                            pu[:, :ft],
                            lhsT=w3s[:, hc, :],
                            rhs=xs_slice(hc, off, off + ft),
                            start=(hc == 0),
                            stop=(hc == HC - 1),
                        )
                    # silu(g) * u  ==  sigmoid(g) * g * u
                    sig = sp.tile([_P, 512], f32, tag="sig", name=f"sig_{ic}_{ti}")
                    nc.scalar.activation(sig[:, :ft], pg[:, :ft], Sigmoid, bias=zbias[:])
                    gs = sp.tile([_P, 512], f32, tag="gs", name=f"gs_{ic}_{ti}")
                    nc.vector.tensor_mul(gs[:, :ft], sig[:, :ft], pg[:, :ft])
                    nc.vector.tensor_mul(
                        hs[:, ic, off : off + ft], gs[:, :ft], pu[:, :ft]
                    )

            # ---- phase 2: y^T[h, t] = sum_i w2[i, h] * h^T[i, t]
            # stationary = w2 sub-blocks [128 (i), 128 (h)], moving = h^T
            # slices; accumulate over i in PSUM with h on partitions.
            w2r = w2.rearrange("(ic p) h -> p ic h", p=_P)
            ICH = IC // 2  # stream w2 per output h-chunk in two half-blocks
            for hc2 in range(HC):
                pys = [
                    pp.tile([_P, 512], f32, tag="ps", name=f"py_{hc2}_{ti}")
                    for ti in range(nt)
                ]
                for half in range(2):
                    w2s = w2p.tile(
                        [_P, ICH, _P], f32r, tag="w2", name=f"w2s_{hc2}_{half}"
                    )
                    # alternate between the two HWDGE rings
                    dma_eng = nc.sync if (2 * hc2 + half) % 2 == 0 else nc.scalar
                    dma_eng.dma_start(
                        w2s[:], w2r[:, half * ICH : (half + 1) * ICH, ts(hc2, _P)]
                    )
                    for ich in range(ICH):
                        ic = half * ICH + ich
                        for ti, (off, ft) in enumerate(zip(foff, fsz)):
                            nc.tensor.matmul(
                                pys[ti][:, :ft],
                                lhsT=w2s[:, ich, :],
                                rhs=hs[:, ic, off : off + ft],
                                start=(ic == 0),
                                stop=(ic == IC - 1),
                            )
                for ti, (off, ft) in enumerate(zip(foff, fsz)):
                    ot = op.tile([_P, 512], f32, tag="ot", name=f"ot_{hc2}_{ti}")
                    nc.vector.tensor_copy(ot[:, :ft], pys[ti][:, :ft])
                    nc.scalar.dma_start(y[ts(hc2, _P), off : off + ft], ot[:, :ft])

    nc.compile()
    return nc


def _get_program(H, I, T):
    key = (H, I, T)
    if key not in _PROG_CACHE:
        _PROG_CACHE[key] = _build_program(H, I, T)
    return _PROG_CACHE[key]


def kernel(x, expert_indices, expert_weights, w1, w2, w3):
    global LAST_RUN
    from concourse.bass_utils import run_bass_kernel_spmd

    x = np.ascontiguousarray(np.asarray(x, dtype=np.float32))
    idx = np.asarray(expert_indices)
    idx_dtype = idx.dtype
    idx = idx.astype(np.int64)
    wts = np.asarray(expert_weights, dtype=np.float32)
    w1 = np.asarray(w1, dtype=np.float32)
    w2 = np.asarray(w2, dtype=np.float32)
    w3 = np.asarray(w3, dtype=np.float32)

    B, H = x.shape
    E, _, I = w1.shape
    assert E == _E, f"expected {_E} experts, got {E}"
    K = idx.shape[1]

    # host-side dispatch: per-token expert weight matrix (merges duplicate
    # top-k hits of the same expert), then token lists per expert
    wmat = np.zeros((B, E), np.float32)
    np.add.at(wmat, (np.arange(B)[:, None], idx), wts)
    sel = np.zeros((B, E), bool)
    sel[np.arange(B)[:, None], idx] = True

    toks = [np.nonzero(sel[:, e])[0] for e in range(E)]
    max_count = max(len(t) for t in toks)

    # capacity per round: SBUF residency (x^T and h^T tiles) caps T
    cap_limit = 608
    rounds = max(1, -(-max_count // cap_limit))
    per_round = -(-max_count // rounds)
    T = max(256, -(-per_round // 16) * 16)

    nc = _get_program(H, I, T)
    xTfull = np.ascontiguousarray(x.T)  # [H, B]

    out = np.zeros((B, H), np.float32)
    for rd in range(rounds):
        in_maps = []
        rtoks = []
        for e in range(E):
            te = toks[e][rd * per_round : (rd + 1) * per_round]
            rtoks.append(te)
            xTe = np.zeros((H, T), np.float32)
            if len(te):
                xTe[:, : len(te)] = xTfull[:, te]
            in_maps.append(
                {
                    "xT": xTe,
                    "w1": np.ascontiguousarray(w1[e]),
                    "w3": np.ascontiguousarray(w3[e]),
                    "w2": np.ascontiguousarray(w2[e]),
                }
            )
        res = run_bass_kernel_spmd(nc, in_maps, list(range(_E)), trace=TRACE)
        LAST_RUN = res
        for e in range(E):
            te = rtoks[e]
            if len(te):
                ye = res.results[e]["y"][:, : len(te)].T  # y^T [H, T] -> [n, H]
                out[te] += wmat[te, e][:, None] * ye

    return out


# revision 19
# speedup vs baseline: 1.1780x; 1.0035x over previous
"""Expert-parallel MoE SwiGLU kernel for Trainium2 (8 NeuronCores).

Strategy: each of the 8 cores owns one expert's weights (w1/w3/w2).  Token
routing (the "all-to-all dispatch") is done host-side: tokens are gathered
per expert, padded to a common capacity T, and each core computes

    y_e = (silu(x_e @ w1_e) * (x_e @ w3_e)) @ w2_e          # [T, H]

for its expert's token set.  The host then scatter-adds the weighted
per-expert outputs back into the [B, H] result.  Matmuls run in float32r
(full-rate fp32 mode on the PE array); all data stays fp32 end to end.
"""

import numpy as np

_P = 128
_E = 8  # experts == cores

# (H, I, T) -> compiled Bass program
_PROG_CACHE = {}
# test hooks: set TRACE=True before calling kernel() to capture an NTFF
# profile; the BassKernelResults of the last run lands in LAST_RUN.
TRACE = False
LAST_RUN = None


def _build_program(H, I, T):
    import concourse.bass as bass
    import concourse.tile as tile
    from concourse import bacc, mybir

    f32 = mybir.dt.float32
    f32r = mybir.dt.float32r
    Sigmoid = mybir.ActivationFunctionType.Sigmoid
    ts = bass.ts

    HC = H // _P
    IC = I // _P
    assert H % _P == 0 and I % _P == 0 and T % 16 == 0

    # token free-dim tiling (both phases): chunks of <=512, as equal as
    # possible (so chunks stay >=256 and f32r matmuls keep 1 cycle/row)
    nt = -(-T // 512)
    q, r = divmod(T, nt)
    fsz = [q + (1 if i < r else 0) for i in range(nt)]
    foff = [sum(fsz[:i]) for i in range(nt)]

    nc = bacc.Bacc(
        "TRN2",
        target_bir_lowering=False,
        debug=False,
        enable_asserts=False,
        num_devices=_E,
    )
    # inputs are declared float32r (same bits as fp32 on the numpy side) so
    # the BIR verifier sees a consistent f32r producer chain into the
    # full-rate f32r matmuls
    xT = nc.dram_tensor("xT", [H, T], f32r, kind="ExternalInput").ap()
    w1 = nc.dram_tensor("w1", [H, I], f32r, kind="ExternalInput").ap()
    w3 = nc.dram_tensor("w3", [H, I], f32r, kind="ExternalInput").ap()
    w2 = nc.dram_tensor("w2", [I, H], f32r, kind="ExternalInput").ap()
    # output is y^T [H, T]: phase 2 accumulates with H on partitions so the
    # token dim needs no 128-granularity (capacity T can hug max_count)
    y = nc.dram_tensor("y", [H, T], f32, kind="ExternalOutput").ap()

    w_bufs = 4  # half-block tiles, 2 i-blocks of lookahead
    w2_bufs = 3

    with tile.TileContext(nc) as tc:
        with (
            tc.tile_pool(name="xp", bufs=1) as xp,
            tc.tile_pool(name="cp", bufs=1) as cp,
            tc.tile_pool(name="wp", bufs=w_bufs) as wp,
            tc.tile_pool(name="w2p", bufs=w2_bufs) as w2p,
            tc.tile_pool(name="hp", bufs=1) as hp,
            tc.tile_pool(name="sp", bufs=3) as sp,
            tc.tile_pool(name="op", bufs=4) as op,
            tc.tile_pool(name="pp", bufs=8, space="PSUM") as pp,
        ):
            zbias = cp.tile([_P, 1], f32)
            nc.any.memset(zbias[:], 0.0)

            # resident activations: x^T as [p, hc, t], h^T as [p, ic, t].
            # x loads as 4 chunks spread over different engine queues so the
            # first chunks land fast and the first accumulation group can
            # start without waiting for the whole 4MB.
            xTr = xT.rearrange("(hc p) t -> p hc t", p=_P)
            n_xc = 4 if HC % 4 == 0 else (2 if HC % 2 == 0 else 1)
            xcs = HC // n_xc
            x_engs = [nc.sync, nc.scalar, nc.gpsimd, nc.gpsimd]
            xs_chunks = []
            for c in range(n_xc):
                xc = xp.tile([_P, xcs, T], f32r, tag=f"xs{c}", name=f"xs_{c}")
                x_engs[c % 4].dma_start(xc[:], xTr[:, c * xcs : (c + 1) * xcs, :])
                xs_chunks.append(xc)

            def xs_slice(hc, lo, hi):
                return xs_chunks[hc // xcs][:, hc % xcs, lo:hi]

            hs = hp.tile([_P, IC, T], f32r)

            w1r = w1.rearrange("(hc p) i -> p hc i", p=_P)
            w3r = w3.rearrange("(hc p) i -> p hc i", p=_P)

            # ---- phase 1: h^T[i, t] = silu(w1^T x)[i, t] * (w3^T x)[i, t]
            # w1/w3 stream per 128-wide i-block, in half-blocks for a finer
            # DMA pipeline (first matmul starts after 512KB, not 1MB)
            HC2 = HC // 2
            for ic in range(IC):
                w1h = []
                w3h = []
                for half in range(2):
                    w1s = wp.tile(
                        [_P, HC2, _P], f32r, tag="w1", name=f"w1s_{ic}_{half}"
                    )
                    nc.sync.dma_start(
                        w1s[:], w1r[:, half * HC2 : (half + 1) * HC2, ts(ic, _P)]
                    )
                    w1h.append(w1s)
                    w3s = wp.tile(
                        [_P, HC2, _P], f32r, tag="w3", name=f"w3s_{ic}_{half}"
                    )
                    nc.scalar.dma_start(
                        w3s[:], w3r[:, half * HC2 : (half + 1) * HC2, ts(ic, _P)]
                    )
                    w3h.append(w3s)
                for ti, (off, ft) in enumerate(zip(foff, fsz)):
                    pg = pp.tile([_P, 512], f32, tag="ps", name=f"pg_{ic}_{ti}")
                    pu = pp.tile([_P, 512], f32, tag="ps", name=f"pu_{ic}_{ti}")
                    for hc in range(HC):
                        nc.tensor.matmul(
                            pg[:, :ft],
                            lhsT=w1h[hc // HC2][:, hc % HC2, :],
                            rhs=xs_slice(hc, off, off + ft),
                            start=(hc == 0),
                            stop=(hc == HC - 1),
                        )
                    for hc in range(HC):
                        nc.tensor.matmul(
                            pu[:, :ft],
                            lhsT=w3h[hc // HC2][:, hc % HC2, :],
                            rhs=xs_slice(hc, off, off + ft),
                            start=(hc == 0),
                            stop=(hc == HC - 1),
                        )
                    # silu(g) * u  ==  sigmoid(g) * g * u
                    sig = sp.tile([_P, 512], f32, tag="sig", name=f"sig_{ic}_{ti}")
                    nc.scalar.activation(sig[:, :ft], pg[:, :ft], Sigmoid, bias=zbias[:])
                    gs = sp.tile([_P, 512], f32, tag="gs", name=f"gs_{ic}_{ti}")
                    nc.vector.tensor_mul(gs[:, :ft], sig[:, :ft], pg[:, :ft])
                    nc.vector.tensor_mul(
                        hs[:, ic, off : off + ft], gs[:, :ft], pu[:, :ft]
                    )

            # ---- phase 2: y^T[h, t] = sum_i w2[i, h] * h^T[i, t]
            # stationary = w2 sub-blocks [128 (i), 128 (h)], moving = h^T
            # slices; accumulate over i in PSUM with h on partitions.
            w2r = w2.rearrange("(ic p) h -> p ic h", p=_P)
            ICH = IC // 2  # stream w2 per output h-chunk in two half-blocks
            for hc2 in range(HC):
                pys = [
                    pp.tile([_P, 512], f32, tag="ps", name=f"py_{hc2}_{ti}")
                    for ti in range(nt)
                ]
                for half in range(2):
                    w2s = w2p.tile(
                        [_P, ICH, _P], f32r, tag="w2", name=f"w2s_{hc2}_{half}"
                    )
                    # alternate between the two HWDGE rings
                    dma_eng = nc.sync if (2 * hc2 + half) % 2 == 0 else nc.scalar
                    dma_eng.dma_start(
                        w2s[:], w2r[:, half * ICH : (half + 1) * ICH, ts(hc2, _P)]
                    )
                    for ich in range(ICH):
                        ic = half * ICH + ich
                        for ti, (off, ft) in enumerate(zip(foff, fsz)):
                            nc.tensor.matmul(
                                pys[ti][:, :ft],
                                lhsT=w2s[:, ich, :],
                                rhs=hs[:, ic, off : off + ft],
                                start=(ic == 0),
                                stop=(ic == IC - 1),
                            )
                for ti, (off, ft) in enumerate(zip(foff, fsz)):
                    ot = op.tile([_P, 512], f32, tag="ot", name=f"ot_{hc2}_{ti}")
                    nc.vector.tensor_copy(ot[:, :ft], pys[ti][:, :ft])
                    nc.scalar.dma_start(y[ts(hc2, _P), off : off + ft], ot[:, :ft])

    nc.compile()
    return nc


def _get_program(H, I, T):
    key = (H, I, T)
    if key not in _PROG_CACHE:
        _PROG_CACHE[key] = _build_program(H, I, T)
    return _PROG_CACHE[key]


def kernel(x, expert_indices, expert_weights, w1, w2, w3):
    global LAST_RUN
    from concourse.bass_utils import run_bass_kernel_spmd

    x = np.ascontiguousarray(np.asarray(x, dtype=np.float32))
    idx = np.asarray(expert_indices)
    idx_dtype = idx.dtype
    idx = idx.astype(np.int64)
    wts = np.asarray(expert_weights, dtype=np.float32)
    w1 = np.asarray(w1, dtype=np.float32)
    w2 = np.asarray(w2, dtype=np.float32)
    w3 = np.asarray(w3, dtype=np.float32)

    B, H = x.shape
    E, _, I = w1.shape
    assert E == _E, f"expected {_E} experts, got {E}"
    K = idx.shape[1]

    # host-side dispatch: per-token expert weight matrix (merges duplicate
    # top-k hits of the same expert), then token lists per expert
    wmat = np.zeros((B, E), np.float32)
    np.add.at(wmat, (np.arange(B)[:, None], idx), wts)
    sel = np.zeros((B, E), bool)
    sel[np.arange(B)[:, None], idx] = True

    toks = [np.nonzero(sel[:, e])[0] for e in range(E)]
    max_count = max(len(t) for t in toks)

    # capacity per round: SBUF residency (x^T and h^T tiles) caps T
    cap_limit = 608
    rounds = max(1, -(-max_count // cap_limit))
    per_round = -(-max_count // rounds)
    T = max(256, -(-per_round // 16) * 16)

    nc = _get_program(H, I, T)
    xTfull = np.ascontiguousarray(x.T)  # [H, B]

    out = np.zeros((B, H), np.float32)
    for rd in range(rounds):
        in_maps = []
        rtoks = []
        for e in range(E):
            te = toks[e][rd * per_round : (rd + 1) * per_round]
            rtoks.append(te)
            xTe = np.zeros((H, T), np.float32)
            if len(te):
                xTe[:, : len(te)] = xTfull[:, te]
            in_maps.append(
                {
                    "xT": xTe,
                    "w1": np.ascontiguousarray(w1[e]),
                    "w3": np.ascontiguousarray(w3[e]),
                    "w2": np.ascontiguousarray(w2[e]),
                }
            )
        res = run_bass_kernel_spmd(nc, in_maps, list(range(_E)), trace=TRACE)
        LAST_RUN = res
        for e in range(E):
            te = rtoks[e]
            if len(te):
                ye = res.results[e]["y"][:, : len(te)].T  # y^T [H, T] -> [n, H]
                out[te] += wmat[te, e][:, None] * ye

    return out


# revision 22
# speedup vs baseline: 1.1907x; 1.0108x over previous
"""Expert-parallel MoE SwiGLU kernel for Trainium2 (8 NeuronCores).

Strategy: each of the 8 cores owns one expert's weights (w1/w3/w2).  Token
routing (the "all-to-all dispatch") is done host-side: tokens are gathered
per expert, padded to a common capacity T, and each core computes

    y_e = (silu(x_e @ w1_e) * (x_e @ w3_e)) @ w2_e          # [T, H]

for its expert's token set.  The host then scatter-adds the weighted
per-expert outputs back into the [B, H] result.  Matmuls run in float32r
(full-rate fp32 mode on the PE array); all data stays fp32 end to end.
"""

import numpy as np

_P = 128
_E = 8  # experts == cores

# (H, I, T) -> compiled Bass program
_PROG_CACHE = {}
# test hooks: set TRACE=True before calling kernel() to capture an NTFF
# profile; the BassKernelResults of the last run lands in LAST_RUN.
TRACE = False
LAST_RUN = None


def _build_program(H, I, T):
    import concourse.bass as bass
    import concourse.tile as tile
    from concourse import bacc, mybir

    f32 = mybir.dt.float32
    f32r = mybir.dt.float32r
    Sigmoid = mybir.ActivationFunctionType.Sigmoid
    ts = bass.ts

    HC = H // _P
    IC = I // _P
    assert H % _P == 0 and I % _P == 0 and T % 16 == 0

    # token free-dim tiling (both phases): chunks of <=512, as equal as
    # possible (so chunks stay >=256 and f32r matmuls keep 1 cycle/row)
    nt = -(-T // 512)
    q, r = divmod(T, nt)
    fsz = [q + (1 if i < r else 0) for i in range(nt)]
    foff = [sum(fsz[:i]) for i in range(nt)]

    nc = bacc.Bacc(
        "TRN2",
        target_bir_lowering=False,
        debug=False,
        enable_asserts=False,
        num_devices=_E,
    )
    # inputs are declared float32r (same bits as fp32 on the numpy side) so
    # the BIR verifier sees a consistent f32r producer chain into the
    # full-rate f32r matmuls
    xT = nc.dram_tensor("xT", [H, T], f32r, kind="ExternalInput").ap()
    w1 = nc.dram_tensor("w1", [H, I], f32r, kind="ExternalInput").ap()
    w3 = nc.dram_tensor("w3", [H, I], f32r, kind="ExternalInput").ap()
    w2 = nc.dram_tensor("w2", [I, H], f32r, kind="ExternalInput").ap()
    # output is y^T [H, T]: phase 2 accumulates with H on partitions so the
    # token dim needs no 128-granularity (capacity T can hug max_count)
    y = nc.dram_tensor("y", [H, T], f32, kind="ExternalOutput").ap()

    # half-block weight tiles: w_bufs//2 i-blocks of DMA lookahead
    w_bufs = 6 if T <= 544 else 4
    w2_bufs = 3

    with tile.TileContext(nc) as tc:
        with (
            tc.tile_pool(name="xp", bufs=1) as xp,
            tc.tile_pool(name="cp", bufs=1) as cp,
            tc.tile_pool(name="wp", bufs=w_bufs) as wp,
            tc.tile_pool(name="w2p", bufs=w2_bufs) as w2p,
            tc.tile_pool(name="hp", bufs=1) as hp,
            tc.tile_pool(name="sp", bufs=2) as sp,
            tc.tile_pool(name="op", bufs=4) as op,
            tc.tile_pool(name="pp", bufs=8, space="PSUM") as pp,
        ):
            zbias = cp.tile([_P, 1], f32)
            nc.any.memset(zbias[:], 0.0)

            # resident activations: x^T as [p, hc, t], h^T as [p, ic, t].
            # x loads as 4 chunks spread over different engine queues so the
            # first chunks land fast and the first accumulation group can
            # start without waiting for the whole 4MB.
            xTr = xT.rearrange("(hc p) t -> p hc t", p=_P)
            n_xc = 2 if HC % 2 == 0 else 1
            xcs = HC // n_xc
            x_engs = [nc.sync, nc.scalar]
            xs_chunks = []
            for c in range(n_xc):
                xc = xp.tile([_P, xcs, T], f32r, tag=f"xs{c}", name=f"xs_{c}")
                x_engs[c % 2].dma_start(xc[:], xTr[:, c * xcs : (c + 1) * xcs, :])
                xs_chunks.append(xc)

            def xs_slice(hc, lo, hi):
                return xs_chunks[hc // xcs][:, hc % xcs, lo:hi]

            hs = hp.tile([_P, IC, T], f32r)

            w1r = w1.rearrange("(hc p) i -> p hc i", p=_P)
            w3r = w3.rearrange("(hc p) i -> p hc i", p=_P)

            # ---- phase 1: h^T[i, t] = silu(w1^T x)[i, t] * (w3^T x)[i, t]
            # w1/w3 stream per 128-wide i-block, in half-blocks for a finer
            # DMA pipeline (first matmul starts after 512KB, not 1MB)
            HC2 = HC // 2
            for ic in range(IC):
                w1h = []
                w3h = []
                for half in range(2):
                    w1s = wp.tile(
                        [_P, HC2, _P], f32r, tag="w1", name=f"w1s_{ic}_{half}"
                    )
                    nc.sync.dma_start(
                        w1s[:], w1r[:, half * HC2 : (half + 1) * HC2, ts(ic, _P)]
                    )
                    w1h.append(w1s)
                    w3s = wp.tile(
                        [_P, HC2, _P], f32r, tag="w3", name=f"w3s_{ic}_{half}"
                    )
                    nc.scalar.dma_start(
                        w3s[:], w3r[:, half * HC2 : (half + 1) * HC2, ts(ic, _P)]
                    )
                    w3h.append(w3s)
                for ti, (off, ft) in enumerate(zip(foff, fsz)):
                    pg = pp.tile([_P, 512], f32, tag="ps", name=f"pg_{ic}_{ti}")
                    pu = pp.tile([_P, 512], f32, tag="ps", name=f"pu_{ic}_{ti}")
                    for hc in range(HC):
                        nc.tensor.matmul(
                            pg[:, :ft],
                            lhsT=w1h[hc // HC2][:, hc % HC2, :],
                            rhs=xs_slice(hc, off, off + ft),
                            start=(hc == 0),
                            stop=(hc == HC - 1),
                        )
                    for hc in range(HC):
                        nc.tensor.matmul(
                            pu[:, :ft],
                            lhsT=w3h[hc // HC2][:, hc % HC2, :],
                            rhs=xs_slice(hc, off, off + ft),
                            start=(hc == 0),
                            stop=(hc == HC - 1),
                        )
                    # silu(g) * u  ==  sigmoid(g) * g * u
                    sig = sp.tile([_P, 512], f32, tag="sig", name=f"sig_{ic}_{ti}")
                    nc.scalar.activation(sig[:, :ft], pg[:, :ft], Sigmoid, bias=zbias[:])
                    gs = sp.tile([_P, 512], f32, tag="gs", name=f"gs_{ic}_{ti}")
                    nc.vector.tensor_mul(gs[:, :ft], sig[:, :ft], pg[:, :ft])
                    nc.vector.tensor_mul(
                        hs[:, ic, off : off + ft], gs[:, :ft], pu[:, :ft]
                    )

            # ---- phase 2: y^T[h, t] = sum_i w2[i, h] * h^T[i, t]
            # stationary = w2 sub-blocks [128 (i), 128 (h)], moving = h^T
            # slices; accumulate over i in PSUM with h on partitions.
            w2r = w2.rearrange("(ic p) h -> p ic h", p=_P)
            ICH = IC // 2  # stream w2 per output h-chunk in two half-blocks
            for hc2 in range(HC):
                pys = [
                    pp.tile([_P, 512], f32, tag="ps", name=f"py_{hc2}_{ti}")
                    for ti in range(nt)
                ]
                for half in range(2):
                    w2s = w2p.tile(
                        [_P, ICH, _P], f32r, tag="w2", name=f"w2s_{hc2}_{half}"
                    )
                    # alternate between the two HWDGE rings
                    dma_eng = nc.sync if (2 * hc2 + half) % 2 == 0 else nc.scalar
                    dma_eng.dma_start(
                        w2s[:], w2r[:, half * ICH : (half + 1) * ICH, ts(hc2, _P)]
                    )
                    for ich in range(ICH):
                        ic = half * ICH + ich
                        for ti, (off, ft) in enumerate(zip(foff, fsz)):
                            nc.tensor.matmul(
                                pys[ti][:, :ft],
                                lhsT=w2s[:, ich, :],
                                rhs=hs[:, ic, off : off + ft],
                                start=(ic == 0),
                                stop=(ic == IC - 1),
                            )
                for ti, (off, ft) in enumerate(zip(foff, fsz)):
                    ot = op.tile([_P, 512], f32, tag="ot", name=f"ot_{hc2}_{ti}")
                    nc.vector.tensor_copy(ot[:, :ft], pys[ti][:, :ft])
                    nc.scalar.dma_start(y[ts(hc2, _P), off : off + ft], ot[:, :ft])

    nc.compile()
    return nc


def _get_program(H, I, T):
    key = (H, I, T)
    if key not in _PROG_CACHE:
        _PROG_CACHE[key] = _build_program(H, I, T)
    return _PROG_CACHE[key]


def kernel(x, expert_indices, expert_weights, w1, w2, w3):
    global LAST_RUN
    from concourse.bass_utils import run_bass_kernel_spmd

    x = np.ascontiguousarray(np.asarray(x, dtype=np.float32))
    idx = np.asarray(expert_indices)
    idx_dtype = idx.dtype
    idx = idx.astype(np.int64)
    wts = np.asarray(expert_weights, dtype=np.float32)
    w1 = np.asarray(w1, dtype=np.float32)
    w2 = np.asarray(w2, dtype=np.float32)
    w3 = np.asarray(w3, dtype=np.float32)

    B, H = x.shape
    E, _, I = w1.shape
    assert E == _E, f"expected {_E} experts, got {E}"
    K = idx.shape[1]

    # host-side dispatch: per-token expert weight matrix (merges duplicate
    # top-k hits of the same expert), then token lists per expert
    wmat = np.zeros((B, E), np.float32)
    np.add.at(wmat, (np.arange(B)[:, None], idx), wts)
    sel = np.zeros((B, E), bool)
    sel[np.arange(B)[:, None], idx] = True

    toks = [np.nonzero(sel[:, e])[0] for e in range(E)]
    max_count = max(len(t) for t in toks)

    # capacity per round: SBUF residency (x^T and h^T tiles) caps T
    cap_limit = 608
    rounds = max(1, -(-max_count // cap_limit))
    per_round = -(-max_count // rounds)
    T = max(256, -(-per_round // 16) * 16)

    nc = _get_program(H, I, T)
    xTfull = np.ascontiguousarray(x.T)  # [H, B]

    out = np.zeros((B, H), np.float32)
    for rd in range(rounds):
        in_maps = []
        rtoks = []
        for e in range(E):
            te = toks[e][rd * per_round : (rd + 1) * per_round]
            rtoks.append(te)
            xTe = np.zeros((H, T), np.float32)
            if len(te):
                xTe[:, : len(te)] = xTfull[:, te]
            in_maps.append(
                {
                    "xT": xTe,
                    "w1": np.ascontiguousarray(w1[e]),
                    "w3": np.ascontiguousarray(w3[e]),
                    "w2": np.ascontiguousarray(w2[e]),
                }
            )
        res = run_bass_kernel_spmd(nc, in_maps, list(range(_E)), trace=TRACE)
        LAST_RUN = res
        for e in range(E):
            te = rtoks[e]
            if len(te):
                ye = res.results[e]["y"][:, : len(te)].T  # y^T [H, T] -> [n, H]
                out[te] += wmat[te, e][:, None] * ye

    return out
